# revision 1
# baseline (speedup 1.0000x reference)
"""Trainium2 Bass kernel for nn_GATModule (GNN message passing / GAT).

Strategy: data-parallel over the batch axis B=4096 across 8 NeuronCores
(512 rows each). Each core sees the full embedding tables in its HBM and
gathers its own neighbor rows with indirect DMAs. No collectives.

Per core, per 128-row batch tile, per side (user / item):
  - gather this/diff_ne/same_ne/diff_rel rows (indirect DMA, 1 idx/partition)
  - same_rel = same_ne * this  (DVE, broadcast AP)
  - keys -> feature-major via PE transposes; pre = Wa_k^T k + (Wa_q^T q + ba)
    accumulated in PSUM (the q term enters via an identity-lhsT matmul with a
    column-replicated AP)
  - tanh on ACT; e = va^T tanh via a replicated-va matmul (b-major columns)
  - e chunks staged to a DRAM tile, read back row-major [b, n]; softmax on
    DVE/ACT; weighted sum of values via TT-mult + strided reduce
  - 2-layer MLP with layernorms, ping-ponging feature-major (matmul) and
    row-major (LN) layouts via PE transposes
"""
import sys
import os

sys.path.insert(0, '/opt/trn_rl_repo')

import numpy as np
from contextlib import ExitStack

import concourse.bass as bass
from concourse import bacc, mybir
from concourse.tile import TileContext
from concourse.masks import make_identity

P = 128          # partitions / batch tile
H = 128          # embedding dim
K = 32           # neighbors per type
NSLOT = 2 * K    # 64 attention slots (0..31 diff, 32..63 same)
NCHUNK = 4       # neighbor slots per 512-col compute chunk
EPS = 1e-5
F32 = mybir.dt.float32
I32 = mybir.dt.int32

NUM_USERS = 100000
NUM_ITEMS = 100000
NUM_PAIRS = 500000
B_FULL = 4096
N_CORES = 8
BC = B_FULL // N_CORES          # rows per core
N_TILES = BC // P               # batch tiles per core


def build_program(n_tiles=N_TILES):
    nc = bacc.Bacc(trn_type="TRN2")

    # ---- DRAM inputs (per-core slices; host reshapes to [n_tiles, ...]) ----
    users_ind = nc.dram_tensor("users_ind", [n_tiles, P], I32, kind="ExternalInput")
    items_ind = nc.dram_tensor("items_ind", [n_tiles, P], I32, kind="ExternalInput")
    u_ne_items = nc.dram_tensor("user_ne_items", [n_tiles, P, K], I32, kind="ExternalInput")
    u_ne_users = nc.dram_tensor("user_ne_users", [n_tiles, P, K], I32, kind="ExternalInput")
    i_ne_users = nc.dram_tensor("item_ne_users", [n_tiles, P, K], I32, kind="ExternalInput")
    i_ne_items = nc.dram_tensor("item_ne_items", [n_tiles, P, K], I32, kind="ExternalInput")
    u_rev = nc.dram_tensor("user_review_inds", [n_tiles, P, K], I32, kind="ExternalInput")
    i_rev = nc.dram_tensor("item_review_inds", [n_tiles, P, K], I32, kind="ExternalInput")
    user_emb = nc.dram_tensor("user_emb", [NUM_USERS, H], F32, kind="ExternalInput")
    item_emb = nc.dram_tensor("item_emb", [NUM_ITEMS, H], F32, kind="ExternalInput")
    review_emb = nc.dram_tensor("review_emb", [NUM_PAIRS, H], F32, kind="ExternalInput")
    Wa = nc.dram_tensor("Wa", [2 * H, H], F32, kind="ExternalInput")
    ba = nc.dram_tensor("ba", [H], F32, kind="ExternalInput")
    va = nc.dram_tensor("va", [H], F32, kind="ExternalInput")
    W1 = nc.dram_tensor("W1", [2 * H, H], F32, kind="ExternalInput")
    b1 = nc.dram_tensor("b1", [H], F32, kind="ExternalInput")
    W2 = nc.dram_tensor("W2", [H, H], F32, kind="ExternalInput")
    b2 = nc.dram_tensor("b2", [H], F32, kind="ExternalInput")
    g1 = nc.dram_tensor("g1", [H], F32, kind="ExternalInput")
    be1 = nc.dram_tensor("be1", [H], F32, kind="ExternalInput")
    g2 = nc.dram_tensor("g2", [H], F32, kind="ExternalInput")
    be2 = nc.dram_tensor("be2", [H], F32, kind="ExternalInput")

    users_pref = nc.dram_tensor("users_pref", [n_tiles, P, H], F32, kind="ExternalOutput")
    items_pref = nc.dram_tensor("items_pref", [n_tiles, P, H], F32, kind="ExternalOutput")
    rel_pref = nc.dram_tensor("relations_pref", [n_tiles, P, H], F32, kind="ExternalOutput")

    AT = mybir.ActivationFunctionType
    ALU = mybir.AluOpType

    def col(dram_vec):
        # [H] dram vector -> [H, 1] AP (one element per partition)
        return dram_vec[:].rearrange("(p o) -> p o", o=1)

    def pbcast(dram_vec, n):
        # [H] dram vector -> [n partitions, H] broadcast AP (partition step 0)
        ap = dram_vec[:]
        return bass.AP(tensor=ap.tensor, offset=ap.offset, ap=[[0, n]] + list(ap.ap))

    with TileContext(nc) as tc:
        with ExitStack() as ctx:
            consts = ctx.enter_context(tc.tile_pool(name="consts", bufs=1))
            idxp = ctx.enter_context(tc.tile_pool(name="idx", bufs=4))
            valsp = ctx.enter_context(tc.tile_pool(name="vals", bufs=2))
            krawp = ctx.enter_context(tc.tile_pool(name="kraw", bufs=4))
            kfmp = ctx.enter_context(tc.tile_pool(name="kfm", bufs=4))
            tanhp = ctx.enter_context(tc.tile_pool(name="tanh", bufs=4))
            esbp = ctx.enter_context(tc.tile_pool(name="esb", bufs=3))
            smallp = ctx.enter_context(tc.tile_pool(name="small", bufs=4))
            tfp = ctx.enter_context(tc.tile_pool(name="tf", bufs=3))
            tmpp = ctx.enter_context(tc.tile_pool(name="tmp", bufs=2))
            outp = ctx.enter_context(tc.tile_pool(name="outp", bufs=2))
            dramp = ctx.enter_context(tc.tile_pool(name="dram", bufs=2, space="DRAM"))
            psp = ctx.enter_context(tc.tile_pool(name="ps", bufs=8, space="PSUM"))

            # ---------------- constants ----------------
            id_sb = consts.tile([P, P], F32)
            make_identity(nc, id_sb[:])
            waq_sb = consts.tile([P, H], F32)
            nc.sync.dma_start(out=waq_sb[:], in_=Wa[0:H, :])
            wak_sb = consts.tile([P, H], F32)
            nc.sync.dma_start(out=wak_sb[:], in_=Wa[H:2 * H, :])
            w1a_sb = consts.tile([P, H], F32)
            nc.sync.dma_start(out=w1a_sb[:], in_=W1[0:H, :])
            w1b_sb = consts.tile([P, H], F32)
            nc.sync.dma_start(out=w1b_sb[:], in_=W1[H:2 * H, :])
            w2_sb = consts.tile([P, H], F32)
            nc.sync.dma_start(out=w2_sb[:], in_=W2[:, :])
            ba_sb = consts.tile([P, 1], F32)
            nc.sync.dma_start(out=ba_sb[:], in_=col(ba))
            b1_sb = consts.tile([P, 1], F32)
            nc.sync.dma_start(out=b1_sb[:], in_=col(b1))
            b2_sb = consts.tile([P, 1], F32)
            nc.sync.dma_start(out=b2_sb[:], in_=col(b2))
            va_sb = consts.tile([P, 1], F32)
            nc.sync.dma_start(out=va_sb[:], in_=col(va))
            va_rep = consts.tile([P, P], F32)
            nc.vector.tensor_copy(out=va_rep[:], in_=va_sb[:, 0:1].to_broadcast([P, P]))
            g1_sb = consts.tile([P, H], F32)
            nc.gpsimd.dma_start(out=g1_sb[:], in_=pbcast(g1, P))
            be1_sb = consts.tile([P, H], F32)
            nc.gpsimd.dma_start(out=be1_sb[:], in_=pbcast(be1, P))
            g2_sb = consts.tile([P, H], F32)
            nc.gpsimd.dma_start(out=g2_sb[:], in_=pbcast(g2, P))
            be2_sb = consts.tile([P, H], F32)
            nc.gpsimd.dma_start(out=be2_sb[:], in_=pbcast(be2, P))
            eps_sb = consts.tile([P, 1], F32)
            nc.vector.memset(eps_sb[:], EPS)

            def pe_warm():
                pass

            def layer_norm_rm(x_rm, g_b, be_b, out_tile):
                """Row-major LN over free dim H. x_rm [P, H] -> out_tile [P, H]."""
                stats = smallp.tile([P, 6], F32, tag="ln_stats")
                nc.vector.bn_stats(out=stats[:], in_=x_rm[:])
                mv = smallp.tile([P, 2], F32, tag="ln_mv")
                nc.vector.bn_aggr(out=mv[:], in_=stats[:])
                sd = smallp.tile([P, 1], F32, tag="ln_sd")
                nc.scalar.activation(out=sd[:], in_=mv[:, 1:2], func=AT.Sqrt,
                                     bias=eps_sb[:, 0:1], scale=1.0)
                rsd = smallp.tile([P, 1], F32, tag="ln_rsd")
                nc.vector.reciprocal(out=rsd[:], in_=sd[:])
                xn = smallp.tile([P, H], F32, tag="ln_xn")
                nc.vector.tensor_scalar(out=xn[:], in0=x_rm[:], scalar1=mv[:, 0:1],
                                        scalar2=rsd[:, 0:1], op0=ALU.subtract,
                                        op1=ALU.mult)
                nc.vector.tensor_tensor(out=xn[:], in0=xn[:], in1=g_b[:], op=ALU.mult)
                nc.vector.tensor_tensor(out=out_tile[:], in0=xn[:], in1=be_b[:], op=ALU.add)

            def transpose128(in_ap, tag):
                """[128,128] SBUF -> [128,128] SBUF transpose via PE + DVE copy."""
                ps = psp.tile([P, 512], F32, tag="ps")
                nc.tensor.transpose(out=ps[:, 0:P], in_=in_ap, identity=id_sb[:])
                sb = tfp.tile([P, P], F32, tag=tag)
                nc.vector.tensor_copy(out=sb[:], in_=ps[:, 0:P])
                return sb

            for t in range(n_tiles):
                u_out_tile = None
                for s in range(2):  # 0 = user side, 1 = item side
                    if s == 0:
                        this_tbl, diff_tbl, same_tbl = user_emb, item_emb, user_emb
                        this_idx = users_ind[t].rearrange("(p o) -> p o", o=1)
                        diff_idx_d, same_idx_d, rev_idx_d = u_ne_items[t], u_ne_users[t], u_rev[t]
                    else:
                        this_tbl, diff_tbl, same_tbl = item_emb, user_emb, item_emb
                        this_idx = items_ind[t].rearrange("(p o) -> p o", o=1)
                        diff_idx_d, same_idx_d, rev_idx_d = i_ne_users[t], i_ne_items[t], i_rev[t]

                    # ---- index tiles ----
                    it_this = idxp.tile([P, 1], I32, tag="it_this")
                    nc.sync.dma_start(out=it_this[:], in_=this_idx)
                    it_diff = idxp.tile([P, K], I32, tag="it_diff")
                    nc.sync.dma_start(out=it_diff[:], in_=diff_idx_d)
                    it_same = idxp.tile([P, K], I32, tag="it_same")
                    nc.sync.dma_start(out=it_same[:], in_=same_idx_d)
                    it_rev = idxp.tile([P, K], I32, tag="it_rev")
                    nc.sync.dma_start(out=it_rev[:], in_=rev_idx_d)

                    # ---- gathers ----
                    this_sb = smallp.tile([P, H], F32, tag="this")
                    nc.gpsimd.indirect_dma_start(
                        out=this_sb[:], out_offset=None, in_=this_tbl[:],
                        in_offset=bass.IndirectOffsetOnAxis(ap=it_this[:, 0:1], axis=0))
                    rev_raw = valsp.tile([P, K * H], F32, tag="rev_raw")
                    for j in range(K):
                        nc.gpsimd.indirect_dma_start(
                            out=rev_raw[:, j * H:(j + 1) * H], out_offset=None,
                            in_=review_emb[:],
                            in_offset=bass.IndirectOffsetOnAxis(ap=it_rev[:, j:j + 1], axis=0))
                    vals_same = valsp.tile([P, K * H], F32, tag="vals_same")
                    for j in range(K):
                        nc.gpsimd.indirect_dma_start(
                            out=vals_same[:, j * H:(j + 1) * H], out_offset=None,
                            in_=same_tbl[:],
                            in_offset=bass.IndirectOffsetOnAxis(ap=it_same[:, j:j + 1], axis=0))
                    vals_diff = valsp.tile([P, K * H], F32, tag="vals_diff")
                    for j in range(K):
                        nc.gpsimd.indirect_dma_start(
                            out=vals_diff[:, j * H:(j + 1) * H], out_offset=None,
                            in_=diff_tbl[:],
                            in_offset=bass.IndirectOffsetOnAxis(ap=it_diff[:, j:j + 1], axis=0))

                    # ---- this^T and q-bias qb = Wa_q^T q + ba ----
                    this_fm = transpose128(this_sb[:], tag="this_fm")
                    qb_ps = psp.tile([P, 512], F32, tag="ps")
                    nc.tensor.matmul(out=qb_ps[:, 0:P], lhsT=waq_sb[:], rhs=this_fm[:],
                                     start=True, stop=True)
                    qb_sb = smallp.tile([P, P], F32, tag="qb")
                    nc.scalar.activation(out=qb_sb[:], in_=qb_ps[:, 0:P],
                                         func=AT.Identity, bias=ba_sb[:, 0:1], scale=1.0)

                    # ---- attention logits e over 16 chunks of 4 slots ----
                    est = dramp.tile([P, NSLOT], F32, tag="e_stage")
                    n_chunks = NSLOT // NCHUNK
                    for c in range(n_chunks):
                        n0 = c * NCHUNK  # absolute slot of first col block
                        if n0 < K:
                            kraw = None  # diff keys live in rev_raw
                        else:
                            kraw = krawp.tile([P, NCHUNK * H], F32, tag="kraw")
                            # keys = same_ne * this
                            m0 = (n0 - K) * H
                            nc.vector.tensor_tensor(
                                out=kraw[:].rearrange("p (n h) -> p n h", n=NCHUNK),
                                in0=vals_same[:, m0:m0 + NCHUNK * H].rearrange(
                                    "p (n h) -> p n h", n=NCHUNK),
                                in1=bass.AP(tensor=this_sb[:].tensor,
                                            offset=this_sb[:].offset,
                                            ap=[list(this_sb[:].ap[0]), [0, NCHUNK],
                                                list(this_sb[:].ap[1])]),
                                op=ALU.mult)
                        # transpose 4x [128,128] into one PSUM bank
                        ksrc = kraw if kraw is not None else rev_raw
                        koff = 0 if kraw is not None else n0 * H
                        kt_ps = psp.tile([P, 512], F32, tag="ps")
                        for j in range(NCHUNK):
                            nc.tensor.transpose(
                                out=kt_ps[:, j * H:(j + 1) * H],
                                in_=ksrc[:, koff + j * H:koff + (j + 1) * H],
                                identity=id_sb[:])
                        k_fm = kfmp.tile([P, NCHUNK * H], F32, tag="kfm")
                        nc.vector.tensor_copy(out=k_fm[:], in_=kt_ps[:, 0:NCHUNK * H])
                        # pre = Wa_k^T k + qb (replicated over the 4 slots)
                        pre_ps = psp.tile([P, 512], F32, tag="ps")
                        nc.tensor.matmul(out=pre_ps[:], lhsT=wak_sb[:], rhs=k_fm[:],
                                         start=True, stop=True)
                        qb_rep = bass.AP(tensor=qb_sb[:].tensor, offset=qb_sb[:].offset,
                                         ap=[list(qb_sb[:].ap[0]), [0, NCHUNK],
                                             list(qb_sb[:].ap[1])])
                        pre_sb = tanhp.tile([P, NCHUNK * H], F32, tag="presb")
                        nc.vector.tensor_tensor(
                            out=pre_sb[:].rearrange("p (n b) -> p n b", n=NCHUNK),
                            in0=pre_ps[:].rearrange("p (n b) -> p n b", n=NCHUNK),
                            in1=qb_rep, op=ALU.add)
                        tanh_sb = tanhp.tile([P, NCHUNK * H], F32, tag="tanh")
                        nc.scalar.activation(out=tanh_sb[:], in_=pre_sb[:],
                                             func=AT.Tanh, bias=0.0, scale=1.0)
                        # e = va^T tanh, b-major columns
                        e_ps = psp.tile([P, 512], F32, tag="ps")
                        nc.tensor.matmul(out=e_ps[:], lhsT=va_rep[:],
                                         rhs=tanh_sb[:].rearrange("p (n b) -> p b n", n=NCHUNK),
                                         start=True, stop=True)
                        pe_warm()
                        e_sb = esbp.tile([P, 512], F32, tag="esb")
                        nc.scalar.activation(out=e_sb[:], in_=e_ps[:], func=AT.Copy,
                                             bias=0.0, scale=1.0)
                        nc.sync.dma_start(out=est[:, n0:n0 + NCHUNK], in_=e_sb[0:1, :])

                    pe_warm()
                    # ---- softmax over 64 slots (row-major) ----
                    e_rm = smallp.tile([P, NSLOT], F32, tag="e_rm")
                    nc.sync.dma_start(out=e_rm[:], in_=est[:])
                    nm = smallp.tile([P, 1], F32, tag="sm_nm")
                    nc.vector.reduce_max(out=nm[:], in_=e_rm[:],
                                         axis=mybir.AxisListType.X, negate=True)
                    p_sb = smallp.tile([P, NSLOT], F32, tag="sm_p")
                    nc.scalar.activation(out=p_sb[:], in_=e_rm[:], func=AT.Exp,
                                         bias=nm[:, 0:1], scale=1.0)
                    ssum = smallp.tile([P, 1], F32, tag="sm_s")
                    nc.vector.reduce_sum(out=ssum[:], in_=p_sb[:], axis=mybir.AxisListType.X)
                    rs = smallp.tile([P, 1], F32, tag="sm_r")
                    nc.vector.reciprocal(out=rs[:], in_=ssum[:])
                    a_sb = smallp.tile([P, NSLOT], F32, tag="sm_a")
                    nc.vector.tensor_scalar_mul(a_sb[:], p_sb[:], rs[:, 0:1])

                    pe_warm()
                    # ---- weighted sum of values ----
                    def wsum_half(vals, a_slice):
                        tmp = tmpp.tile([P, K * H], F32, tag="wsum_tmp")
                        nc.vector.tensor_tensor(
                            out=tmp[:].rearrange("p (n h) -> p n h", n=K),
                            in0=vals[:].rearrange("p (n h) -> p n h", n=K),
                            in1=a_slice.to_broadcast([P, K, H]),
                            op=ALU.mult)
                        w = K * H
                        while w > H:
                            w //= 2
                            nc.vector.tensor_tensor(out=tmp[:, :w], in0=tmp[:, :w],
                                                    in1=tmp[:, w:2 * w], op=ALU.add)
                        return tmp
                    td = wsum_half(vals_diff, a_sb[:, 0:K])
                    ts_ = wsum_half(vals_same, a_sb[:, K:NSLOT])
                    pref = smallp.tile([P, H], F32, tag="pref")
                    nc.vector.tensor_tensor(out=pref[:], in0=td[:, :H], in1=ts_[:, :H],
                                            op=ALU.add)

                    # ---- transform MLP ----
                    pref_fm = transpose128(pref[:], tag="pref_fm")
                    l1_ps = psp.tile([P, 512], F32, tag="ps")
                    nc.tensor.matmul(out=l1_ps[:, 0:P], lhsT=w1a_sb[:], rhs=this_fm[:],
                                     start=True, stop=False)
                    nc.tensor.matmul(out=l1_ps[:, 0:P], lhsT=w1b_sb[:], rhs=pref_fm[:],
                                     start=False, stop=True)
                    x1_fm = tfp.tile([P, P], F32, tag="x1_fm")
                    nc.scalar.activation(out=x1_fm[:], in_=l1_ps[:, 0:P], func=AT.Relu,
                                         bias=b1_sb[:, 0:1], scale=1.0)
                    x1_rm = transpose128(x1_fm[:], tag="x1_rm")
                    x1_ln = tfp.tile([P, P], F32, tag="x1_ln")
                    layer_norm_rm(x1_rm, g1_sb, be1_sb, x1_ln)
                    x1_ln_fm = transpose128(x1_ln[:], tag="x1_ln_fm")
                    l2_ps = psp.tile([P, 512], F32, tag="ps")
                    nc.tensor.matmul(out=l2_ps[:, 0:P], lhsT=w2_sb[:], rhs=x1_ln_fm[:],
                                     start=True, stop=True)
                    x2_fm = tfp.tile([P, P], F32, tag="x2_fm")
                    nc.scalar.activation(out=x2_fm[:], in_=l2_ps[:, 0:P], func=AT.Relu,
                                         bias=b2_sb[:, 0:1], scale=1.0)
                    x2_rm = transpose128(x2_fm[:], tag="x2_rm")
                    out_rm = outp.tile([P, H], F32, tag=("u_out" if s == 0 else "i_out"))
                    layer_norm_rm(x2_rm, g2_sb, be2_sb, out_rm)

                    if s == 0:
                        u_out_tile = out_rm
                        nc.sync.dma_start(out=users_pref[t], in_=out_rm[:])
                    else:
                        nc.sync.dma_start(out=items_pref[t], in_=out_rm[:])
                        rel = outp.tile([P, H], F32, tag="rel_out")
                        nc.vector.tensor_tensor(out=rel[:], in0=u_out_tile[:],
                                                in1=out_rm[:], op=ALU.mult)
                        nc.sync.dma_start(out=rel_pref[t], in_=rel[:])

    nc.finalize()
    return nc


_PROGRAM_CACHE = {}


def _get_program(n_tiles=N_TILES):
    if n_tiles not in _PROGRAM_CACHE:
        _PROGRAM_CACHE[n_tiles] = build_program(n_tiles)
    return _PROGRAM_CACHE[n_tiles]


def run(inputs, trace=False):
    """inputs: dict of FULL-size numpy arrays. Returns (res_tuple, exec_time_ns)."""
    from concourse.bass_utils import run_bass_kernel_spmd

    nc = _get_program(N_TILES)
    shared = {k: np.asarray(inputs[k]) for k in
              ("user_emb", "item_emb", "review_emb", "Wa", "ba", "va", "W1",
               "b1", "W2", "b2", "g1", "be1", "g2", "be2")}
    in_maps = []
    for c in range(N_CORES):
        sl = slice(c * BC, (c + 1) * BC)
        m = dict(shared)
        m["users_ind"] = np.asarray(inputs["users_ind"][sl]).reshape(N_TILES, P)
        m["items_ind"] = np.asarray(inputs["items_ind"][sl]).reshape(N_TILES, P)
        for k in ("user_ne_items", "user_ne_users", "item_ne_users",
                  "item_ne_items", "user_review_inds", "item_review_inds"):
            m[k] = np.asarray(inputs[k][sl]).reshape(N_TILES, P, K)
        in_maps.append(m)

    res = run_bass_kernel_spmd(nc, in_maps, list(range(N_CORES)), trace=trace)
    ups, ips, rps = [], [], []
    for c in range(N_CORES):
        ups.append(res.results[c]["users_pref"].reshape(BC, H))
        ips.append(res.results[c]["items_pref"].reshape(BC, H))
        rps.append(res.results[c]["relations_pref"].reshape(BC, H))
    out = (np.concatenate(ups), np.concatenate(ips), np.concatenate(rps))
    return out, res.exec_time_ns


def kernel(**inputs):
    out, _ = run(inputs, trace=False)
    return out

